# revision 1
# baseline (speedup 1.0000x reference)
"""Contrastive-loss kernel for Trainium2 (8 NeuronCores, data-parallel).

Reference computation (per batch row b):
    samples = concat([positives[b, -1], negatives[b]])        # [129, 1024]
    sim[s]  = <a_b, samples[s]> / (max(|a_b|,eps) * max(|samples[s]|,eps))
    loss_b  = logsumexp(sim) - sim[0]
    loss    = mean_b loss_b

Sharding: batch dim (2048) split across 8 cores (256 rows each).
Per core layout: batch on SBUF partitions (2 groups of 128), D on free dim.
Hot loop per sample index s (one [128, 1024] pass each):
    DVE scalar_tensor_tensor: out = samp*anchor, accum -> dot[:, s]
    ACT activation(Square):   out = samp^2,      accum -> ns2[:, s]
Negatives stream in [128, 4, 1024] 2 MiB chunks (contiguous per partition).
accum_out writes are NOT tracked by the Tile scheduler (observed races on
HW), so explicit add_dep_helper edges order them before their readers.
The [128, 129] softmax epilogue runs on-chip; each core outputs 128x2
per-row losses, summed/averaged on host (the allreduce-mean equivalent).
"""

import numpy as np
from contextlib import ExitStack

import concourse.bass as bass
import concourse.tile as tile
import concourse.mybir as mybir
from concourse import bacc
from concourse.bass_utils import run_bass_kernel_spmd
from concourse.tile_rust import add_dep_helper

F32 = mybir.dt.float32
ALU = mybir.AluOpType
AF = mybir.ActivationFunctionType

N_CORES = 8
B = 2048
B_LOC = B // N_CORES          # 256
D = 1024
N_NEG = 128
S = N_NEG + 1                 # 129 logits per row; s=0 is the positive
EPS = 1e-6
S_CHUNK = 8                   # sample-indices per DMA (4 MiB transfers)


def _build_nc(
    b_loc: int = B_LOC,
    n_neg: int = N_NEG,
    s_chunk: int = S_CHUNK,
    bulk_bufs: int = 4,
    n_iters: int = 1,
    dma_engines=("sync",),
    interleave: bool = True,
) -> bass.Bass:
    n_groups = b_loc // 128
    s_tot = n_neg + 1
    nc = bacc.Bacc("TRN2", target_bir_lowering=False)
    anchor = nc.dram_tensor("anchor", [b_loc, D], F32, kind="ExternalInput")
    pos = nc.dram_tensor("pos", [b_loc, D], F32, kind="ExternalInput")
    neg = nc.dram_tensor("neg", [b_loc, n_neg, D], F32, kind="ExternalInput")
    out = nc.dram_tensor("loss_cols", [128, n_groups], F32, kind="ExternalOutput")
    engs = [getattr(nc, e) for e in dma_engines]

    with tile.TileContext(nc) as tc, ExitStack() as ctx:
        bulk = ctx.enter_context(tc.tile_pool(name="bulk", bufs=bulk_bufs))
        apool = ctx.enter_context(tc.tile_pool(name="apool", bufs=2))
        tscr = ctx.enter_context(tc.tile_pool(name="tscr", bufs=2))
        ascr = ctx.enter_context(tc.tile_pool(name="ascr", bufs=2))
        persist = ctx.enter_context(tc.tile_pool(name="persist", bufs=1))
        sm = ctx.enter_context(tc.tile_pool(name="sm", bufs=1))

        def dot_accum(samp_ap, a_tile, accum_col):
            sv = tscr.tile([128, D], F32, tag="tscr")
            return nc.vector.scalar_tensor_tensor(
                out=sv[:], in0=samp_ap, scalar=1.0, in1=a_tile[:],
                op0=ALU.mult, op1=ALU.mult, accum_out=accum_col,
            )

        def sq_accum(samp_ap, accum_col):
            sa = ascr.tile([128, D], F32, tag="ascr")
            return nc.scalar.activation(
                out=sa[:], in_=samp_ap, func=AF.Square, accum_out=accum_col
            )

        def phase2(g, loss_cols, dot_all, ns2_all, na2, dps, nps, na2_prod):
            ns_ = sm.tile([128, s_tot], F32, tag=f"ns{g}")
            i_ns = nc.scalar.activation(out=ns_[:], in_=ns2_all[:], func=AF.Sqrt)
            for p in nps:
                add_dep_helper(i_ns.ins, p.ins, reason="accum ns2 -> sqrt")
            na_ = sm.tile([128, 1], F32, tag=f"na{g}")
            i_na = nc.scalar.activation(out=na_[:], in_=na2[:], func=AF.Sqrt)
            add_dep_helper(i_na.ins, na2_prod.ins, reason="accum na2 -> sqrt")
            nc.vector.tensor_scalar_max(ns_[:], ns_[:], EPS)
            nc.vector.tensor_scalar_max(na_[:], na_[:], EPS)
            denom = sm.tile([128, s_tot], F32, tag=f"den{g}")
            nc.vector.tensor_scalar_mul(denom[:], ns_[:], na_[:])
            inv = sm.tile([128, s_tot], F32, tag=f"inv{g}")
            nc.vector.reciprocal(out=inv[:], in_=denom[:])
            sim = sm.tile([128, s_tot], F32, tag=f"sim{g}")
            i_sim = nc.vector.tensor_mul(sim[:], dot_all[:], inv[:])
            for p in dps:
                add_dep_helper(i_sim.ins, p.ins, reason="accum dot -> sim")
            # |sim| <= 1, so exp never overflows: no max-subtraction needed
            e = sm.tile([128, s_tot], F32, tag=f"e{g}")
            sumexp = sm.tile([128, 1], F32, tag=f"se{g}")
            i_exp = nc.scalar.activation(
                out=e[:], in_=sim[:], func=AF.Exp, accum_out=sumexp[:]
            )
            lse = sm.tile([128, 1], F32, tag=f"lse{g}")
            i_ln = nc.scalar.activation(out=lse[:], in_=sumexp[:], func=AF.Ln)
            add_dep_helper(i_ln.ins, i_exp.ins, reason="accum sumexp -> ln")
            nc.vector.tensor_sub(loss_cols[:, g : g + 1], lse[:], sim[:, 0:1])

        for _it in range(n_iters):
            loss_cols = sm.tile([128, n_groups], F32, tag="loss_cols")
            dots, ns2s, na2s = [], [], []
            dot_prods, ns2_prods, na2_prods = [], [], []
            for g in range(n_groups):
                b0 = g * 128
                a_tile = apool.tile([128, D], F32, tag="a_tile")
                nc.sync.dma_start(out=a_tile[:], in_=anchor[b0 : b0 + 128, :])

                dot_all = persist.tile([128, s_tot], F32, tag=f"dot{g}")
                ns2_all = persist.tile([128, s_tot], F32, tag=f"ns2{g}")
                na2 = persist.tile([128, 1], F32, tag=f"na2{g}")
                dps, nps = [], []

                na2_prods.append(sq_accum(a_tile[:], na2[:]))

                # s = 0: the (last) positive
                p_tile = apool.tile([128, D], F32, tag="p_tile")
                nc.sync.dma_start(out=p_tile[:], in_=pos[b0 : b0 + 128, :])
                dps.append(dot_accum(p_tile[:], a_tile, dot_all[:, 0:1]))
                nps.append(sq_accum(p_tile[:], ns2_all[:, 0:1]))

                # s = 1..n_neg: negatives, streamed in chunks of s_chunk
                for c in range(n_neg // s_chunk):
                    nt = bulk.tile([128, s_chunk, D], F32, tag="negchunk")
                    engs[c % len(engs)].dma_start(
                        out=nt[:],
                        in_=neg[b0 : b0 + 128, c * s_chunk : (c + 1) * s_chunk, :],
                    )
                    for j in range(s_chunk):
                        s = 1 + c * s_chunk + j
                        dps.append(dot_accum(nt[:, j, :], a_tile, dot_all[:, s : s + 1]))
                        nps.append(sq_accum(nt[:, j, :], ns2_all[:, s : s + 1]))
                dots.append(dot_all)
                ns2s.append(ns2_all)
                na2s.append(na2)
                dot_prods.append(dps)
                ns2_prods.append(nps)
                if interleave:
                    phase2(g, loss_cols, dot_all, ns2_all, na2, dps, nps,
                           na2_prods[g])

            # ---- softmax epilogue (emitted here unless interleaved) ----
            if not interleave:
                for g in range(n_groups):
                    phase2(g, loss_cols, dots[g], ns2s[g], na2s[g],
                           dot_prods[g], ns2_prods[g], na2_prods[g])
            nc.sync.dma_start(out=out[:], in_=loss_cols[:])
    nc.finalize()
    return nc


_NC_CACHE = None


def _get_nc() -> bass.Bass:
    global _NC_CACHE
    if _NC_CACHE is None:
        _NC_CACHE = _build_nc()
    return _NC_CACHE


def _make_in_maps(anchor, positives, negatives):
    anchor = np.asarray(anchor)
    positives = np.asarray(positives)
    negatives = np.asarray(negatives)
    in_maps = []
    for i in range(N_CORES):
        sl = slice(i * B_LOC, (i + 1) * B_LOC)
        in_maps.append(
            {
                "anchor": np.ascontiguousarray(anchor[sl, 0, :], dtype=np.float32),
                "pos": np.ascontiguousarray(positives[sl, -1, :], dtype=np.float32),
                "neg": np.ascontiguousarray(negatives[sl], dtype=np.float32),
            }
        )
    return in_maps


def _reduce_results(results):
    total = 0.0
    for r in results:
        total += float(np.asarray(r["loss_cols"], dtype=np.float64).sum())
    return np.array(total / B, dtype=np.float32)


def run_sharded(anchor, positives, negatives, **spmd_kwargs):
    """Run on 8 cores; returns (loss_scalar, BassKernelResults)."""
    nc = _get_nc()
    in_maps = _make_in_maps(anchor, positives, negatives)
    res = run_bass_kernel_spmd(nc, in_maps, core_ids=list(range(N_CORES)), **spmd_kwargs)
    return _reduce_results(res.results), res


def kernel(anchor, positives, negatives):
    loss, _ = run_sharded(anchor, positives, negatives)
    return loss



# revision 17
# speedup vs baseline: 2.3279x; 2.3279x over previous
"""Contrastive-loss kernel for Trainium2 (8 NeuronCores, data-parallel).

Reference computation (per batch row b):
    samples = concat([positives[b, -1], negatives[b]])        # [129, 1024]
    sim[s]  = <a_b, samples[s]> / (max(|a_b|,eps) * max(|samples[s]|,eps))
    loss_b  = logsumexp(sim) - sim[0]
    loss    = mean_b loss_b

Sharding: batch dim (2048) split across 8 cores (256 rows each).
Per core layout: batch on SBUF partitions (2 groups of 128), D on free dim.

All inputs are cast to bf16 on the host before upload (the f32 pipeline
was DMA/DVE-bound; tolerance 2e-2 leaves orders of magnitude of margin).
The two elementwise passes over the 129x1024 samples are split across
three engines, tuned empirically via slope timing:

  * dots  <a, s>: samples [0, pe_s) go to the TENSOR engine via a
    redundant matmul in a host-transposed layout: for each d-block of
    128, out[b', (s, b)] = sum_d a[b', d] * samp[b, s, d] accumulated
    over 8 d-blocks into PSUM [128, 512] tiles (4 samples each, 4 tags x
    2 bufs = all 8 banks); the wanted diagonal b' == b is extracted by a
    DVE mult-with-identity + accum_out into dot[:, s]. Remaining samples
    use DVE scalar_tensor_tensor (bf16 2x mode), accum_out -> dot[:, s].
  * squares s^2 (for ||s||): ACT activation(Square) with accum_out ->
    ns2[:, s]; every sq_mod-th square runs on DVE instead -- keeping the
    two instruction streams interleaved measures far faster than either
    pure split (engine queue-depth effects).

Negatives stream in [128, s_chunk, 1024] bf16 chunks; the transposed
copy for the TensorE samples streams as [128, 8, 2048] band tiles
(DMA bandwidth is nowhere near binding; double-shipping those samples
is free). accum_out writes are NOT tracked by the Tile scheduler
(observed races on HW), so explicit add_dep_helper edges order them
before their readers. The [128, 129] softmax epilogue runs on-chip in
f32; each core outputs 128x2 per-row losses, summed/averaged on host
(the allreduce-mean equivalent).

Measured (slope method, 8 cores): f32 baseline 284.7 us -> bf16 134.7 us
-> hybrid TensorE+DVE/ACT 53.7 us.
"""

import numpy as np
from contextlib import ExitStack

import concourse.bass as bass
import concourse.tile as tile
import concourse.mybir as mybir
from concourse import bacc
from concourse.bass_utils import run_bass_kernel_spmd
from concourse.tile_rust import add_dep_helper

F32 = mybir.dt.float32
BF16 = mybir.dt.bfloat16
NP_BF16 = mybir.dt.np(mybir.dt.bfloat16)
ALU = mybir.AluOpType
AF = mybir.ActivationFunctionType

N_CORES = 8
B = 2048
B_LOC = B // N_CORES          # 256
D = 1024
N_NEG = 128
S = N_NEG + 1                 # 129 logits per row; s=0 is the positive
EPS = 1e-6
S_CHUNK = 8                   # sample-indices per DMA (2 MiB bf16 transfers)


def _build_nc(
    b_loc: int = B_LOC,
    n_neg: int = N_NEG,
    s_chunk: int = S_CHUNK,
    bulk_bufs: int = 4,
    tscr_bufs: int = 2,
    ascr_bufs: int = 2,
    sq_first: bool = False,
    n_iters: int = 1,
    dma_engines=("sync",),
    interleave: bool = True,
    sq_mod: int = 1000000,     # every sq_mod-th square runs on DVE
    sq_res: int = -1,
    dot_mod: int = 1000000,    # every dot_mod-th dot runs on GpSimd
    dot_res: int = -1,
    pe_s: int = 0,             # samples [0, pe_s) get dots via TensorE
    band_bufs: int = 2,
    ablate: str = "none",      # none | dma_only | dots_only | squares_only
) -> bass.Bass:
    n_groups = b_loc // 128
    s_tot = n_neg + 1
    assert pe_s % 16 == 0
    n_bands = pe_s // 16
    nc = bacc.Bacc("TRN2", target_bir_lowering=False)
    anchor = nc.dram_tensor("anchor", [b_loc, D], BF16, kind="ExternalInput")
    pos = nc.dram_tensor("pos", [b_loc, D], BF16, kind="ExternalInput")
    neg = nc.dram_tensor("neg", [b_loc, n_neg, D], BF16, kind="ExternalInput")
    out = nc.dram_tensor("loss_cols", [128, n_groups], F32, kind="ExternalOutput")
    if pe_s:
        samt = nc.dram_tensor(
            "samt", [n_groups, n_bands, 128, 8, 16 * 128], BF16,
            kind="ExternalInput",
        )
        at = nc.dram_tensor("at", [128, n_groups, 8, 128], BF16,
                            kind="ExternalInput")
        ident = nc.dram_tensor("ident", [128, 128], F32, kind="ExternalInput")
    engs = [getattr(nc, e) for e in dma_engines]

    with tile.TileContext(nc) as tc, ExitStack() as ctx:
        bulk = ctx.enter_context(tc.tile_pool(name="bulk", bufs=bulk_bufs))
        apool = ctx.enter_context(tc.tile_pool(name="apool", bufs=2))
        tscr = ctx.enter_context(tc.tile_pool(name="tscr", bufs=tscr_bufs))
        ascr = ctx.enter_context(tc.tile_pool(name="ascr", bufs=ascr_bufs))
        persist = ctx.enter_context(tc.tile_pool(name="persist", bufs=1))
        sm = ctx.enter_context(tc.tile_pool(name="sm", bufs=1))
        if pe_s:
            bandp = ctx.enter_context(tc.tile_pool(name="bandp", bufs=band_bufs))
            psum = ctx.enter_context(tc.tile_pool(name="psum", bufs=2, space="PSUM"))
            scrap = ctx.enter_context(tc.tile_pool(name="scrap", bufs=2))

        def dot_accum(s, samp_ap, a_tile, accum_col):
            sv = tscr.tile([128, D], BF16, tag="tscr")
            eng = nc.gpsimd if s % dot_mod == dot_res else nc.vector
            return eng.scalar_tensor_tensor(
                out=sv[:], in0=samp_ap, scalar=1.0, in1=a_tile[:],
                op0=ALU.mult, op1=ALU.mult, accum_out=accum_col,
            )

        def sq_accum_act(samp_ap, accum_col):
            sa = ascr.tile([128, D], BF16, tag="ascr")
            return nc.scalar.activation(
                out=sa[:], in_=samp_ap, func=AF.Square, accum_out=accum_col
            )

        def sq_accum_dve(samp_ap, accum_col):
            sa = tscr.tile([128, D], BF16, tag="tscr")
            return nc.vector.scalar_tensor_tensor(
                out=sa[:], in0=samp_ap, scalar=1.0, in1=samp_ap,
                op0=ALU.mult, op1=ALU.mult, accum_out=accum_col,
            )

        def sq_accum(s, samp_ap, accum_col):
            if s % sq_mod == sq_res:
                return sq_accum_dve(samp_ap, accum_col)
            return sq_accum_act(samp_ap, accum_col)

        do_dots = ablate in ("none", "dots_only")
        do_sqs = ablate in ("none", "squares_only")

        def phase2(g, loss_cols, dot_all, ns2_all, na2, dps, nps, na2_prod):
            ns_ = sm.tile([128, s_tot], F32, tag=f"ns{g}")
            i_ns = nc.scalar.activation(out=ns_[:], in_=ns2_all[:], func=AF.Sqrt)
            for p in nps:
                add_dep_helper(i_ns.ins, p.ins, reason="accum ns2 -> sqrt")
            na_ = sm.tile([128, 1], F32, tag=f"na{g}")
            i_na = nc.scalar.activation(out=na_[:], in_=na2[:], func=AF.Sqrt)
            add_dep_helper(i_na.ins, na2_prod.ins, reason="accum na2 -> sqrt")
            nc.vector.tensor_scalar_max(ns_[:], ns_[:], EPS)
            nc.vector.tensor_scalar_max(na_[:], na_[:], EPS)
            denom = sm.tile([128, s_tot], F32, tag=f"den{g}")
            nc.vector.tensor_scalar_mul(denom[:], ns_[:], na_[:])
            inv = sm.tile([128, s_tot], F32, tag=f"inv{g}")
            nc.vector.reciprocal(out=inv[:], in_=denom[:])
            sim = sm.tile([128, s_tot], F32, tag=f"sim{g}")
            i_sim = nc.vector.tensor_mul(sim[:], dot_all[:], inv[:])
            for p in dps:
                add_dep_helper(i_sim.ins, p.ins, reason="accum dot -> sim")
            # |sim| <= 1, so exp never overflows: no max-subtraction needed
            e = sm.tile([128, s_tot], F32, tag=f"e{g}")
            sumexp = sm.tile([128, 1], F32, tag=f"se{g}")
            i_exp = nc.scalar.activation(
                out=e[:], in_=sim[:], func=AF.Exp, accum_out=sumexp[:]
            )
            lse = sm.tile([128, 1], F32, tag=f"lse{g}")
            i_ln = nc.scalar.activation(out=lse[:], in_=sumexp[:], func=AF.Ln)
            add_dep_helper(i_ln.ins, i_exp.ins, reason="accum sumexp -> ln")
            nc.vector.tensor_sub(loss_cols[:, g : g + 1], lse[:], sim[:, 0:1])

        for _it in range(n_iters):
            loss_cols = sm.tile([128, n_groups], F32, tag="loss_cols")
            if pe_s:
                at_tile = persist.tile([128, n_groups, 8, 128], BF16, tag="at")
                nc.sync.dma_start(out=at_tile[:], in_=at[:])
                ident_tile = persist.tile([128, 128], F32, tag="ident")
                nc.sync.dma_start(out=ident_tile[:], in_=ident[:])
            dots, ns2s, na2s = [], [], []
            dot_prods, ns2_prods, na2_prods = [], [], []
            for g in range(n_groups):
                b0 = g * 128
                a_tile = apool.tile([128, D], BF16, tag="a_tile")
                nc.sync.dma_start(out=a_tile[:], in_=anchor[b0 : b0 + 128, :])

                dot_all = persist.tile([128, s_tot], F32, tag=f"dot{g}")
                ns2_all = persist.tile([128, s_tot], F32, tag=f"ns2{g}")
                na2 = persist.tile([128, 1], F32, tag=f"na2{g}")
                dps, nps = [], []
                if not do_dots:
                    nc.gpsimd.memset(dot_all[:], 0.0)
                if not do_sqs:
                    nc.gpsimd.memset(ns2_all[:], 0.0)

                na2_prods.append(sq_accum_act(a_tile[:], na2[:]))

                # --- TensorE dots for samples [0, pe_s) ---
                if pe_s and do_dots:
                    for band in range(n_bands):
                        bt = bandp.tile([128, 8, 16 * 128], BF16, tag="band")
                        nc.sync.dma_start(out=bt[:], in_=samt[g, band])
                        for t in range(4):
                            ps = psum.tile([128, 512], F32, tag=f"ps{t}")
                            for dblk in range(8):
                                nc.tensor.matmul(
                                    ps[:],
                                    lhsT=at_tile[:, g, dblk, :],
                                    rhs=bt[:, dblk, t * 512:(t + 1) * 512],
                                    start=(dblk == 0), stop=(dblk == 7),
                                )
                            for sl in range(4):
                                s = band * 16 + t * 4 + sl
                                sc = scrap.tile([128, 128], F32, tag="scrap")
                                dps.append(nc.vector.scalar_tensor_tensor(
                                    out=sc[:],
                                    in0=ps[:, sl * 128:(sl + 1) * 128],
                                    scalar=1.0, in1=ident_tile[:],
                                    op0=ALU.mult, op1=ALU.mult,
                                    accum_out=dot_all[:, s:s + 1],
                                ))

                # s = 0: the (last) positive
                p_tile = apool.tile([128, D], BF16, tag="p_tile")
                nc.sync.dma_start(out=p_tile[:], in_=pos[b0 : b0 + 128, :])
                if do_dots and pe_s == 0:
                    dps.append(dot_accum(0, p_tile[:], a_tile, dot_all[:, 0:1]))
                if do_sqs:
                    nps.append(sq_accum(0, p_tile[:], ns2_all[:, 0:1]))

                # s = 1..n_neg: negatives, streamed in chunks of s_chunk
                for c in range(n_neg // s_chunk):
                    nt = bulk.tile([128, s_chunk, D], BF16, tag="negchunk")
                    engs[c % len(engs)].dma_start(
                        out=nt[:],
                        in_=neg[b0 : b0 + 128, c * s_chunk : (c + 1) * s_chunk, :],
                    )
                    for j in range(s_chunk):
                        s = 1 + c * s_chunk + j
                        if sq_first:
                            if do_sqs:
                                nps.append(sq_accum(s, nt[:, j, :], ns2_all[:, s : s + 1]))
                            if do_dots and s >= pe_s:
                                dps.append(dot_accum(s, nt[:, j, :], a_tile, dot_all[:, s : s + 1]))
                        else:
                            if do_dots and s >= pe_s:
                                dps.append(dot_accum(s, nt[:, j, :], a_tile, dot_all[:, s : s + 1]))
                            if do_sqs:
                                nps.append(sq_accum(s, nt[:, j, :], ns2_all[:, s : s + 1]))
                dots.append(dot_all)
                ns2s.append(ns2_all)
                na2s.append(na2)
                dot_prods.append(dps)
                ns2_prods.append(nps)
                if interleave:
                    phase2(g, loss_cols, dot_all, ns2_all, na2, dps, nps,
                           na2_prods[g])

            # ---- softmax epilogue (emitted here unless interleaved) ----
            if not interleave:
                for g in range(n_groups):
                    phase2(g, loss_cols, dots[g], ns2s[g], na2s[g],
                           dot_prods[g], ns2_prods[g], na2_prods[g])
            nc.sync.dma_start(out=out[:], in_=loss_cols[:])
    nc.finalize()
    return nc


_NC_CACHE = None


def _get_nc() -> bass.Bass:
    global _NC_CACHE
    if _NC_CACHE is None:
        _NC_CACHE = _build_nc(**BUILD_KW)
    return _NC_CACHE


def _make_in_maps(anchor, positives, negatives, pe_s=0):
    anchor = np.asarray(anchor)
    positives = np.asarray(positives)
    negatives = np.asarray(negatives)
    n_bands = pe_s // 16
    ident = np.eye(128, dtype=np.float32)
    in_maps = []
    for i in range(N_CORES):
        sl = slice(i * B_LOC, (i + 1) * B_LOC)
        a_core = np.ascontiguousarray(anchor[sl, 0, :]).astype(NP_BF16)
        p_core = np.ascontiguousarray(positives[sl, -1, :]).astype(NP_BF16)
        n_core = np.ascontiguousarray(negatives[sl]).astype(NP_BF16)
        m = {"anchor": a_core, "pos": p_core, "neg": n_core}
        if pe_s:
            # samples[b, s, d]: s=0 is the positive, 1..128 the negatives
            samples = np.concatenate([p_core[:, None, :], n_core], axis=1)
            # samt[g, band, dd, dblk, s_loc, b] =
            #     samples[128g + b, 16*band + s_loc, 128*dblk + dd]
            samt = np.empty((2, n_bands, 128, 8, 16 * 128), dtype=NP_BF16)
            for g in range(2):
                sub = samples[128 * g : 128 * (g + 1)]        # [128b, 129s, 1024d]
                for band in range(n_bands):
                    arr = sub[:, band * 16 : (band + 1) * 16, :]   # [b, s, d]
                    arr = arr.transpose(2, 1, 0)                   # [d, s, b]
                    arr = arr.reshape(8, 128, 16, 128)             # [dblk, dd, s, b]
                    arr = arr.transpose(1, 0, 2, 3)                # [dd, dblk, s, b]
                    samt[g, band] = arr.reshape(128, 8, 16 * 128)
            # at[dd, g, dblk, b] = anchor[128g + b, 128*dblk + dd]
            a4 = a_core.reshape(2, 128, 8, 128)                    # [g, b, dblk, dd]
            m["at"] = np.ascontiguousarray(a4.transpose(3, 0, 2, 1))
            m["samt"] = samt
            m["ident"] = ident
        in_maps.append(m)
    return in_maps


def _reduce_results(results):
    total = 0.0
    for r in results:
        total += float(np.asarray(r["loss_cols"], dtype=np.float64).sum())
    return np.array(total / B, dtype=np.float32)


# Best measured config: TensorE covers dots for samples [0, 96); every 6th
# square runs on DVE (keeps the ACT queue interleaved); rest of squares on ACT.
PE_S = 48
BUILD_KW = {"pe_s": PE_S, "sq_mod": 8}


def run_sharded(anchor, positives, negatives, **spmd_kwargs):
    """Run on 8 cores; returns (loss_scalar, BassKernelResults)."""
    nc = _get_nc()
    in_maps = _make_in_maps(anchor, positives, negatives, pe_s=PE_S)
    res = run_bass_kernel_spmd(nc, in_maps, core_ids=list(range(N_CORES)), **spmd_kwargs)
    return _reduce_results(res.results), res


def kernel(anchor, positives, negatives):
    loss, _ = run_sharded(anchor, positives, negatives)
    return loss
